# revision 82
# baseline (speedup 1.0000x reference)
"""Trainium2 Bass kernel for channel attention (XCA-style) nn.Module.

One image per NeuronCore (batch=8 over 8 cores). Pipeline per image:
  qkv 1x1 conv (matmul) -> 3x3 depthwise conv -> l2norm channel attention
  -> (attn @ v folded with proj 1x1 into a single matmul).

Key structure:
  * Stripes of S rows with 1-row halos; stripe tiles use a 132-element
    padded row stride [p p x0..x127 p p] so conv taps never wrap rows.
  * Gram matrix q@k^T accumulated UNNORMALIZED over stripes in PSUM,
    alternating two PSUM banks per pixel column to break the
    accumulate-to-same-address RAW hazard; normalization afterwards via
    the gram diagonal.
  * attn@v + proj collapse into y = M^T.T @ v with per-head
    M^T_h = A_h^T @ proj_w^T[48h:48h+48, :].
  * Depthwise conv: 5 taps on the PE as diagonal matmuls accumulated in
    PSUM (evacuated as the accumulator init); 4 taps on the DVE as
    tensor_scalar muls (4x mode) + tree-structured tensor adds (2x) so
    only the final add depends on the PSUM evacuation.
  * All transposes on the PE (no DMA transposes); gram matmuls
    interleaved with the next group's transposes.
  * PSUM evacuations spread across Scalar/Vector/GpSimd engines.
  * Channels head-interleaved [q_h0,k_h0,...,q_h3,k_h3, v] so per-head
    gram operands are contiguous column slices of the transposed stripe.
"""

import numpy as np
import ml_dtypes

import concourse.bass as bass
import concourse.tile as tile
from concourse import mybir, bacc
from concourse.bass_utils import run_bass_kernel_spmd

F32 = mybir.dt.float32
BF16 = mybir.dt.bfloat16
FP8 = mybir.dt.float8e4
DR = mybir.MatmulPerfMode.DoubleRow
AX = mybir.AxisListType
OP = mybir.AluOpType
ACTF = mybir.ActivationFunctionType

C, H, W = 192, 128, 128
HW = H * W
HEADS, CH = 4, 48
RS = 132                      # padded row stride
TAPS = [(dy, dx) for dy in (-1, 0, 1) for dx in (-1, 0, 1)]
SA = 32
SB = 32
# per-chunk PE-tap candidates (rest go to the DVE). Normal stripes use
# the first 5; the last stripe of each pass uses all 6 so the PE (idle
# at the pass tail) relieves the DVE backlog that gates the tail.
CHUNK_PE_TAPS = [[0, 2, 6, 8, 3, 1]] * 5
DIAG_OFF = [0]
for _t in CHUNK_PE_TAPS:
    DIAG_OFF.append(DIAG_OFF[-1] + len(_t))
N_DIAG = DIAG_OFF[-1]

_cached = {}


def _build_program():
    nc = bacc.Bacc("TRN2", target_bir_lowering=False, debug=False, num_devices=8)

    xq_d = nc.dram_tensor("xq", [96, 2, HW], FP8, kind="ExternalInput").ap()
    xb_d = nc.dram_tensor("xb", [C, H, W], BF16, kind="ExternalInput").ap()
    w1q_d = nc.dram_tensor("w1q", [96, 2, 576], FP8, kind="ExternalInput").ap()
    wvt_d = nc.dram_tensor("wvt", [C, 192], BF16, kind="ExternalInput").ap()
    dwt_d = nc.dram_tensor("dwt", [128, 5, 18], F32, kind="ExternalInput").ap()
    pwt_d = nc.dram_tensor("pwt", [48, 4, C], BF16, kind="ExternalInput").ap()
    i96_d = nc.dram_tensor("i96", [96, 96], F32, kind="ExternalInput").ap()
    i128_d = nc.dram_tensor("i128", [128, 128], BF16, kind="ExternalInput").ap()
    t4_d = nc.dram_tensor("t4", [1, 4], F32, kind="ExternalInput").ap()
    y_d = nc.dram_tensor("y", [C, HW], F32, kind="ExternalOutput").ap()

    with tile.TileContext(nc) as tc:
        _emit(nc, tc, xq_d, xb_d, w1q_d, wvt_d, dwt_d, pwt_d, i96_d,
              i128_d, t4_d, y_d)
    nc.finalize()
    return nc


def _emit(nc, tc, xq_d, xb_d, w1q_d, wvt_d, dwt_d, pwt_d, i96_d,
          i128_d, t4_d, y_d):
    from contextlib import ExitStack

    with ExitStack() as top:
        persist = top.enter_context(tc.tile_pool(name="persist", bufs=1))

        w1q = persist.tile([96, 2, 576], FP8)
        nc.sync.dma_start(out=w1q, in_=w1q_d)
        dwt = persist.tile([128, 5, 18], F32)
        nc.sync.dma_start(out=dwt, in_=dwt_d)
        i128 = persist.tile([128, 128], BF16)
        nc.sync.dma_start(out=i128, in_=i128_d)

        # first x stripe as early as possible, 4-way split across DMA
        # queues (all engines idle at startup), ahead of the weights
        # only needed later
        xq = persist.tile([96, 2, HW], FP8)
        hpx = SA * W // 2
        pxa, pxb = slice(0, hpx), slice(hpx, 2 * hpx)
        nc.sync.dma_start(out=xq[:, 0, pxa], in_=xq_d[:, 0, pxa])
        nc.scalar.dma_start(out=xq[:, 0, pxb], in_=xq_d[:, 0, pxb])
        nc.gpsimd.dma_start(out=xq[:, 1, pxa], in_=xq_d[:, 1, pxa])
        nc.gpsimd.dma_start(out=xq[:, 1, pxb], in_=xq_d[:, 1, pxb])

        wvt0 = persist.tile([128, 192], BF16)
        wvt1 = persist.tile([64, 192], BF16)
        nc.sync.dma_start(out=wvt0, in_=wvt_d[0:128, :])
        nc.sync.dma_start(out=wvt1, in_=wvt_d[128:192, :])
        pwt = persist.tile([48, 4, C], BF16)
        nc.sync.dma_start(out=pwt, in_=pwt_d)
        i96 = persist.tile([96, 96], F32)
        nc.sync.dma_start(out=i96, in_=i96_d)
        t4s = persist.tile([1, 4], F32)
        nc.sync.dma_start(out=t4s, in_=t4_d)
        # broadcast temperature to 48 partitions early (off critical path)
        t4b = persist.tile([48, HEADS], F32)
        nc.sync.dma_start(
            out=t4b,
            in_=bass.AP(tensor=t4_d.tensor, offset=t4_d.offset,
                        ap=[[0, 48], [1, HEADS]]))

        # per-(chunk, pe-tap) diagonal weight matrices
        diag = persist.tile([128, N_DIAG, 128], BF16)
        for ci in range(5):
            for j, t in enumerate(CHUNK_PE_TAPS[ci]):
                nc.vector.tensor_scalar_mul(diag[:, DIAG_OFF[ci] + j, :],
                                            i128, dwt[:, ci, t:t + 1])

        mta = persist.tile([128, C], BF16)
        mtb = persist.tile([64, C], BF16)

        gpool = top.enter_context(tc.tile_pool(name="gpool", bufs=1, space="PSUM"))
        g_ps = [gpool.tile([96, HEADS, 96], F32, name=f"gps{i}")
                for i in range(2)]

        def load_stripe(s):
            # split the two K-slabs across DMA queues to halve load latency
            px = slice(s * SA * W, (s + 1) * SA * W)
            nc.sync.dma_start(out=xq[:, 0, px], in_=xq_d[:, 0, px])
            nc.gpsimd.dma_start(out=xq[:, 1, px], in_=xq_d[:, 1, px])

        # ================= pass A: q,k =================
        with ExitStack() as pa:
            pre_p = pa.enter_context(tc.tile_pool(name="pre_p", bufs=4))
            tmp_p = pa.enter_context(tc.tile_pool(name="tmp_p", bufs=4))
            acc_p = pa.enter_context(tc.tile_pool(name="acc_p", bufs=4))
            qkt_p = pa.enter_context(tc.tile_pool(name="qkt_p", bufs=2))
            ps_p = pa.enter_context(tc.tile_pool(name="ps_p", bufs=2, space="PSUM"))
            cps_p = pa.enter_context(tc.tile_pool(name="cps_p", bufs=2, space="PSUM"))
            tps_p = pa.enter_context(tc.tile_pool(name="tps_p", bufs=2, space="PSUM"))

            for s in range(H // SA):
                if s + 1 < H // SA:
                    load_stripe(s + 1)
                _conv_stripe_qk(nc, s, SA, w1q, xq,
                                dwt, diag, i128,
                                pre_p, tmp_p, acc_p, qkt_p, ps_p, cps_p,
                                tps_p, g_ps, n_stripes=H // SA)

        # ============ pass B: v + output, finalize overlapped ============
        with ExitStack() as pb:
            xbs_p = pb.enter_context(tc.tile_pool(name="xbs_p", bufs=2))
            pre_p = pb.enter_context(tc.tile_pool(name="preb_p", bufs=2))
            tmp_p = pb.enter_context(tc.tile_pool(name="tmpb_p", bufs=4))
            acc_p = pb.enter_context(tc.tile_pool(name="accb_p", bufs=2))
            y_p = pb.enter_context(tc.tile_pool(name="y_p", bufs=3))
            ps_p = pb.enter_context(tc.tile_pool(name="psb_p", bufs=2, space="PSUM"))
            cps_p = pb.enter_context(tc.tile_pool(name="cpsb_p", bufs=2, space="PSUM"))

            def load_vstripe(s):
                r0 = s * SB
                lo, hi = max(r0 - 1, 0), min(r0 + SB + 1, H)
                xs0 = xbs_p.tile([128, SB + 2, W], BF16, tag="xs0", name="xs0")
                xs1 = xbs_p.tile([64, SB + 2, W], BF16, tag="xs1", name="xs1")
                nc.sync.dma_start(out=xs0[:, 0:hi - lo, :],
                                  in_=xb_d[0:128, lo:hi, :])
                nc.gpsimd.dma_start(out=xs1[:, 0:hi - lo, :],
                                    in_=xb_d[128:192, lo:hi, :])
                return (xs0, xs1)

            # stripe 0 v-production first, so the finalize (which only uses
            # ACT/DVE + tiny PE ops) overlaps with PE's qkv/tap work.
            xbs = {0: load_vstripe(0)}
            _conv_stripe_v_front(nc, 0, SB, wvt0, wvt1, xbs[0], dwt, diag,
                                 pre_p, tmp_p, acc_p, ps_p, cps_p,
                                 n_stripes=H // SB)
            xbs[1] = load_vstripe(1)

            with ExitStack() as fz:
                _finalize(nc, tc, fz, g_ps, i96, t4b, pwt, mta, mtb)

            yps_p = pb.enter_context(tc.tile_pool(name="yps_p", bufs=2, space="PSUM"))

            accs = {}
            for s in range(H // SB):
                if s > 0:
                    accs[s] = _conv_stripe_v_front(
                        nc, s, SB, wvt0, wvt1, xbs[s], dwt, diag,
                        pre_p, tmp_p, acc_p, ps_p, cps_p, n_stripes=H // SB)
                else:
                    accs[s] = _LAST_V_ACC[0]
                if s + 2 < H // SB:
                    xbs[s + 2] = load_vstripe(s + 2)
                _stripe_y(nc, s, SB, accs[s], mta, mtb, yps_p, y_p, y_d,
                          last=(s == H // SB - 1))


_LAST_V_ACC = [None]


def _pre_pads(nc, p, s, S, n_stripes, n_bufs):
    if s < n_bufs:  # pads once per pool buffer slot
        nc.gpsimd.memset(p[:, :, 0:2], 0.0)
        nc.gpsimd.memset(p[:, :, 130:132], 0.0)
    if s == 0:
        nc.gpsimd.memset(p[:, 0, :], 0.0)
    if s == n_stripes - 1:
        nc.gpsimd.memset(p[:, S + 1, :], 0.0)


def _stripe_matmul_dr(nc, s, S, n_stripes, oc_defs, w1q, xq,
                      pre, ps_p, n_bufs):
    """1x1 conv (fp8 DoubleRow, K=192 in one pass) into padded
    [p, S+2, RS] stripe tiles (halo rows included)."""
    r0 = s * S
    lo, hi = max(r0 - 1, 0), min(r0 + S + 1, H)
    for i, (ocp, ocsl) in enumerate(oc_defs):
        p = pre[i]
        _pre_pads(nc, p, s, S, n_stripes, n_bufs)
        # rows [lo, hi) of the image, in groups of <=4 rows (512 px)
        r = lo
        while r < hi:
            nr = min(4, hi - r)
            ps = ps_p.tile([ocp, 512], F32, tag="mmps", name="mmps")
            px = slice(r * W, (r + nr) * W)
            nc.tensor.matmul(ps[:, 0:nr * W], w1q[:, :, ocsl], xq[:, :, px],
                             start=True, stop=True, perf_mode=DR)
            tr = r - (r0 - 1)
            nc.scalar.copy(
                p[:, tr:tr + nr, 2:130],
                ps[:, 0:nr * W].rearrange("p (a b) -> p a b", b=W))
            r += nr


def _stripe_matmul_bf(nc, s, S, n_stripes, oc_defs, wvt0, wvt1, xbs,
                      pre, ps_p, n_bufs):
    """bf16 1x1 conv for the v chunks, from streamed x tiles
    (xbs = (xs0 [128, 34, W], xs1 [64, 34, W]) holding rows [lo, hi))."""
    r0 = s * S
    lo, hi = max(r0 - 1, 0), min(r0 + S + 1, H)
    xs0, xs1 = xbs
    for i, (ocp, ocsl) in enumerate(oc_defs):
        p = pre[i]
        _pre_pads(nc, p, s, S, n_stripes, n_bufs)
        r = lo
        while r < hi:
            nr = min(4, hi - r)
            ps = ps_p.tile([ocp, 512], F32, tag="mmps", name="mmps")
            rl = r - lo
            nc.tensor.matmul(ps[:, 0:nr * W], wvt0[:, ocsl],
                             xs0[:, rl:rl + nr, :], start=True, stop=False)
            nc.tensor.matmul(ps[:, 0:nr * W], wvt1[:, ocsl],
                             xs1[:, rl:rl + nr, :], start=False, stop=True)
            tr = r - (r0 - 1)
            nc.scalar.copy(
                p[:, tr:tr + nr, 2:130],
                ps[:, 0:nr * W].rearrange("p (a b) -> p a b", b=W))
            r += nr


def _dw_conv(nc, pre, tmp_p, acc, dwt, diag, oc_list, S, cps_p, pe_n=5):
    """3x3 depthwise conv on padded [p, S+2, RS] tiles -> acc [p, S, W].
    Per-chunk PE taps run as diagonal matmuls into PSUM (evacuated as
    the accumulator init); DVE taps are 4x-mode muls + tree adds."""
    for i, oc in enumerate(oc_list):
        p, a = pre[i], acc[i]
        np_ = p.shape[0]
        pe_taps = CHUNK_PE_TAPS[oc][:pe_n]
        dve_taps = [t for t in range(9) if t not in pe_taps]
        npe = len(pe_taps)
        # process 4-row groups in pairs, interleaving the two PSUM banks:
        # same-bank matmuls are then 2 apart, which hides the
        # accumulate-to-same-address RAW between consecutive taps (the
        # same trick as the even/odd gram split). Also halves stationary
        # switches per matmul pair.
        for g2 in range(S // 8):
            cpA = cps_p.tile([np_, 512], F32, tag="cps", name="cpsA")
            cpB = cps_p.tile([np_, 512], F32, tag="cps", name="cpsB")
            for j, t in enumerate(pe_taps):
                dy, dx = TAPS[t]
                for cp, g in ((cpA, 2 * g2), (cpB, 2 * g2 + 1)):
                    rhs = p[:, 1 + dy + 4 * g: 1 + dy + 4 * g + 4,
                            2 + dx: 130 + dx]
                    nc.tensor.matmul(cp, diag[:np_, DIAG_OFF[oc] + j, :np_],
                                     rhs, start=(j == 0),
                                     stop=(j == npe - 1),
                                     skip_group_check=True)
            for cp, g in ((cpA, 2 * g2), (cpB, 2 * g2 + 1)):
                nc.scalar.copy(a[:, 4 * g:4 * g + 4, :],
                               cp.rearrange("p (a b) -> p a b", b=W))
        # DVE taps: muls into tmp tiles (4x mode), then a tree of adds so
        # only the final add depends on the PSUM evacuation above.
        tms = []
        for t in dve_taps:
            dy, dx = TAPS[t]
            wv = dwt[:np_, oc, t:t + 1]
            v = p[:, 1 + dy: 1 + dy + S, 2 + dx: 130 + dx]
            tm = tmp_p.tile([128, S, W], BF16, tag="tmp", name="tmp")
            nc.vector.tensor_scalar_mul(tm[:np_], v, wv)
            tms.append(tm[:np_])
        n = len(tms)
        nc.vector.tensor_add(tms[0], tms[0], tms[1])
        if n >= 4:
            nc.vector.tensor_add(tms[2], tms[2], tms[3])
        if n >= 3:
            nc.vector.tensor_add(tms[0], tms[0], tms[2])
        if n == 5:
            nc.vector.tensor_add(tms[0], tms[0], tms[4])
        nc.vector.tensor_add(a, a, tms[0])


def _conv_stripe_qk(nc, s, S, w1q, xq, dwt, diag, i128,
                    pre_p, tmp_p, acc_p, qkt_p, ps_p, cps_p, tps_p, g_ps,
                    n_stripes):
    pre = [pre_p.tile([128, S + 2, RS], BF16, tag="pre", name=f"pre{i}")
           for i in range(3)]
    oc_defs = [(128, slice(0, 128)), (128, slice(128, 256)),
               (128, slice(256, 384))]
    _stripe_matmul_dr(nc, s, S, n_stripes, oc_defs, w1q, xq,
                      pre, ps_p, n_bufs=4)

    acc = [acc_p.tile([128, S, W], BF16, tag="acc", name=f"acc{i}")
           for i in range(3)]
    _dw_conv(nc, pre, tmp_p, acc, dwt, diag, [0, 1, 2], S, cps_p,
             pe_n=6 if s == n_stripes - 1 else 5)

    # transpose each 128-px row to [px, ch]; interleave gram matmuls of
    # the previous 8-row group with this group's transposes.
    qkt = qkt_p.tile([128, S, 384], BF16)
    first = (s == 0)
    last = (s == n_stripes - 1)
    evac = [nc.scalar.copy, nc.vector.tensor_copy, nc.vector.tensor_copy]

    def emit_gram(g):
        for k in range(8):
            pc = 8 * g + k
            gp = g_ps[pc % 2]
            for h in range(HEADS):
                nc.tensor.matmul(
                    gp[:, h, :], qkt[:, pc, 96 * h:96 * h + 96],
                    qkt[:, pc, 96 * h:96 * h + 96],
                    start=(first and pc < 2),
                    stop=(last and pc >= S - 2),
                    skip_group_check=True)

    for g in range(S // 8):
        for i in range(3):
            tps = tps_p.tile([128, 8, 128], BF16, tag="tps", name="tps")
            for k in range(8):
                nc.tensor.transpose(tps[:, k, :], acc[i][:, 8 * g + k, :],
                                    i128)
            evac[i](qkt[:, 8 * g:8 * g + 8, 128 * i:128 * (i + 1)], tps)
        if g > 0:
            emit_gram(g - 1)
    emit_gram(S // 8 - 1)


def _conv_stripe_v_front(nc, s, S, wvt0, wvt1, xbs, dwt, diag,
                         pre_p, tmp_p, acc_p, ps_p, cps_p, n_stripes):
    """qkv 1x1 + depthwise for the v chunks of stripe s; returns acc."""
    pre = [pre_p.tile([128, S + 2, RS], BF16, tag="prev", name="prev0"),
           pre_p.tile([64, S + 2, RS], BF16, tag="prev1", name="prev1")]
    oc_defs = [(128, slice(0, 128)), (64, slice(128, 192))]
    _stripe_matmul_bf(nc, s, S, n_stripes, oc_defs, wvt0, wvt1, xbs,
                      pre, ps_p, n_bufs=2)

    acc = [acc_p.tile([128, S, W], BF16, tag="accv", name="accv0"),
           acc_p.tile([64, S, W], BF16, tag="accv1", name="accv1")]
    _dw_conv(nc, pre, tmp_p, acc, dwt, diag, [3, 4], S, cps_p,
             pe_n=6 if s == n_stripes - 1 else 5)
    _LAST_V_ACC[0] = acc
    return acc


def _stripe_y(nc, s, S, acc, mta, mtb, yps_p, y_p, y_d, last=False):
    """y = M^T.T @ v  (attn+proj folded)"""
    r0 = s * S
    for g in range(S // 4):
        pxs = slice(4 * g, 4 * g + 4)
        dpx = slice(r0 * W + 512 * g, r0 * W + 512 * (g + 1))
        yp0 = yps_p.tile([128, 512], F32, tag="yp", name="yp0")
        nc.tensor.matmul(yp0, mta[:, 0:128], acc[0][:, pxs, :],
                         start=True, stop=False)
        nc.tensor.matmul(yp0, mtb[:, 0:128], acc[1][:, pxs, :],
                         start=False, stop=True)
        y0 = y_p.tile([128, 512], F32, tag="y0", name="y0")
        nc.scalar.copy(y0, yp0)
        nc.sync.dma_start(out=y_d[0:128, dpx], in_=y0)
        yp1 = yps_p.tile([64, 512], F32, tag="yp", name="yp1")
        nc.tensor.matmul(yp1, mta[:, 128:192], acc[0][:, pxs, :],
                         start=True, stop=False)
        nc.tensor.matmul(yp1, mtb[:, 128:192], acc[1][:, pxs, :],
                         start=False, stop=True)
        y1 = y_p.tile([64, 512], F32, tag="y1", name="y1")
        if last:  # parallelize the drain of the final stripe
            nc.vector.tensor_copy(y1, yp1)
        else:
            nc.scalar.copy(y1, yp1)
        nc.sync.dma_start(out=y_d[128:192, dpx], in_=y1)


def _finalize(nc, tc, fz, g_ps, i96, t4b, pwt, mta, mtb):
    """gram -> l2-normalized attention -> softmax -> folded M^T."""
    fpool = fz.enter_context(tc.tile_pool(name="fpool", bufs=1))
    fps = fz.enter_context(tc.tile_pool(name="fps", bufs=1, space="PSUM"))

    gs0 = fpool.tile([96, HEADS, 96], F32)
    nc.scalar.copy(gs0, g_ps[0])
    gs1 = fpool.tile([96, HEADS, 96], F32)
    nc.vector.tensor_copy(gs1, g_ps[1])
    gs = fpool.tile([96, HEADS, 96], F32)
    nc.vector.tensor_add(gs, gs0, gs1)

    i96b = bass.AP(tensor=i96.tensor, offset=i96.offset,
                   ap=[list(i96.ap[0]), [0, HEADS], [1, 96]])
    gdiag = fpool.tile([96, HEADS, 96], F32)
    nc.vector.tensor_mul(gdiag, gs, i96b)
    nrm2 = fpool.tile([96, HEADS], F32)
    nc.vector.reduce_sum(nrm2, gdiag, axis=AX.X)
    nrm = fpool.tile([96, HEADS], F32)
    nc.scalar.activation(nrm, nrm2, ACTF.Sqrt)
    nc.vector.tensor_scalar_max(nrm, nrm, 1e-12)
    rstd = fpool.tile([96, HEADS], F32)
    nc.vector.reciprocal(rstd, nrm)

    rq = fpool.tile([48, HEADS], F32)
    nc.vector.tensor_mul(rq, rstd[0:48, :], t4b)

    rkk = fpool.tile([48, HEADS], F32)
    nc.sync.dma_start(out=rkk, in_=rstd[48:96, :])
    rkps = fps.tile([4, 48], F32)
    nc.tensor.transpose(rkps, rkk, i96[0:48, 0:48])
    rkrow = fpool.tile([4, 48], F32)
    nc.vector.tensor_copy(rkrow, rkps)
    dram_p = fz.enter_context(tc.tile_pool(name="dram_p", bufs=1,
                                           space="DRAM"))
    rkd = dram_p.tile([4, 48], F32)
    nc.sync.dma_start(out=rkd, in_=rkrow)
    rk = fpool.tile([48, HEADS, 48], F32)
    for h in range(HEADS):
        bsrc = bass.AP(tensor=rkd.tensor,
                       offset=rkd.offset + h * 48,
                       ap=[[0, 48], [1, 48]])
        nc.sync.dma_start(out=rk[:, h, :], in_=bsrc)

    z = fpool.tile([48, HEADS, 48], F32)
    for h in range(HEADS):
        nc.vector.scalar_tensor_tensor(
            out=z[:, h, :], in0=gs[0:48, h, 48:96],
            scalar=rq[:, h:h + 1], in1=rk[:, h, :],
            op0=OP.mult, op1=OP.mult)
    mx = fpool.tile([48, HEADS], F32)
    nc.vector.reduce_max(mx, z, axis=AX.X)
    nmx = fpool.tile([48, HEADS], F32)
    nc.vector.tensor_scalar_mul(nmx, mx, -1.0)
    ez = fpool.tile([48, HEADS, 48], F32)
    for h in range(HEADS):
        nc.scalar.activation(ez[:, h, :], z[:, h, :], ACTF.Exp,
                             bias=nmx[:, h:h + 1], scale=1.0)
    sm = fpool.tile([48, HEADS], F32)
    nc.vector.reduce_sum(sm, ez, axis=AX.X)
    rs = fpool.tile([48, HEADS], F32)
    nc.vector.reciprocal(rs, sm)
    a_bf = fpool.tile([48, HEADS, 48], BF16)
    for h in range(HEADS):
        nc.vector.tensor_scalar_mul(a_bf[:, h, :], ez[:, h, :],
                                    rs[:, h:h + 1])

    m_bf = fpool.tile([48, HEADS, C], BF16)
    for h in range(HEADS):
        mps = fps.tile([48, C], F32, tag="mps", name="mps")
        nc.tensor.matmul(mps, a_bf[:, h, :], pwt[:, h, :],
                         start=True, stop=True)
        nc.scalar.copy(m_bf[:, h, :], mps)

    nc.sync.dma_start(out=mta[0:48, :], in_=m_bf[:, 0, :])
    nc.sync.dma_start(out=mta[48:96, :], in_=m_bf[:, 1, :])
    nc.sync.dma_start(out=mta[96:128, :], in_=m_bf[0:32, 2, :])
    nc.sync.dma_start(out=mtb[0:16, :], in_=m_bf[32:48, 2, :])
    nc.sync.dma_start(out=mtb[16:64, :], in_=m_bf[:, 3, :])


# ---------------- host glue ----------------

def _host_inputs(x, qkv_w, dw_w, proj_w, temperature):
    perm = []
    for h in range(HEADS):
        perm += list(range(h * CH, (h + 1) * CH))
        perm += list(range(C + h * CH, C + (h + 1) * CH))
    perm += list(range(2 * C, 3 * C))
    perm = np.array(perm)

    w1 = np.asarray(qkv_w)[perm]
    w1t = np.ascontiguousarray(w1.T).astype(np.float32)  # [192, 576]
    w1q = np.stack([w1t[0:96], w1t[96:192]], axis=1)     # [96, 2, 576]
    w1q = np.clip(w1q, -240.0, 240.0).astype(ml_dtypes.float8_e4m3)
    wvt = np.ascontiguousarray(w1t[:, 384:576]).astype(ml_dtypes.bfloat16)
    dw = np.asarray(dw_w)[perm, 0]
    dwt = np.zeros((128, 5, 18), np.float32)
    for ci in range(5):
        rows = min(128, 576 - ci * 128)
        taps = dw[ci * 128: ci * 128 + rows].reshape(rows, 9)
        dwt[:rows, ci, 0:9] = taps
        dwt[:rows, ci, 9:18] = -taps
    pT = np.asarray(proj_w).T.astype(np.float32)
    pwt = np.stack([pT[48 * h:48 * (h + 1)] for h in range(4)],
                   axis=1).astype(ml_dtypes.bfloat16)
    i96 = np.eye(96, dtype=np.float32)
    i128 = np.eye(128, dtype=ml_dtypes.bfloat16)
    t4 = np.asarray(temperature).reshape(1, HEADS).astype(np.float32)
    shared = {
        "w1q": w1q, "wvt": wvt, "dwt": dwt, "pwt": pwt, "i96": i96,
        "i128": i128, "t4": t4,
    }
    xs = np.asarray(x).reshape(8, C, HW).astype(np.float32)
    xqs = np.clip(xs, -240.0, 240.0)
    xqs = np.stack([xqs[:, 0:96], xqs[:, 96:192]], axis=2) \
        .astype(ml_dtypes.float8_e4m3)                   # [8, 96, 2, HW]
    xbs = xs.astype(ml_dtypes.bfloat16).reshape(8, C, H, W)
    return shared, xqs, xbs


def kernel(x, qkv_w, dw_w, proj_w, temperature, _trace=False):
    if "nc" not in _cached:
        _cached["nc"] = _build_program()
    nc = _cached["nc"]
    shared, xqs, xbs = _host_inputs(x, qkv_w, dw_w, proj_w, temperature)
    in_maps = [dict(shared, xq=np.ascontiguousarray(xqs[i]),
                    xb=np.ascontiguousarray(xbs[i]))
               for i in range(8)]
    res = run_bass_kernel_spmd(nc, in_maps, core_ids=list(range(8)),
                               trace=_trace)
    out = np.stack([np.asarray(res.results[i]["y"]).reshape(C, H, W)
                    for i in range(8)])
    if _trace:
        _cached["last_exec_time_ns"] = res.exec_time_ns
        _cached["last_results"] = res
    return out


# revision 84
# speedup vs baseline: 1.0152x; 1.0152x over previous
"""Trainium2 Bass kernel for channel attention (XCA-style) nn.Module.

One image per NeuronCore (batch=8 over 8 cores). Pipeline per image:
  qkv 1x1 conv (matmul) -> 3x3 depthwise conv -> l2norm channel attention
  -> (attn @ v folded with proj 1x1 into a single matmul).

Key structure:
  * Stripes of S rows with 1-row halos; stripe tiles use a 132-element
    padded row stride [p p x0..x127 p p] so conv taps never wrap rows.
  * Gram matrix q@k^T accumulated UNNORMALIZED over stripes in PSUM,
    alternating two PSUM banks per pixel column to break the
    accumulate-to-same-address RAW hazard; normalization afterwards via
    the gram diagonal.
  * attn@v + proj collapse into y = M^T.T @ v with per-head
    M^T_h = A_h^T @ proj_w^T[48h:48h+48, :].
  * Depthwise conv: 5 taps on the PE as diagonal matmuls accumulated in
    PSUM (evacuated as the accumulator init); 4 taps on the DVE as
    tensor_scalar muls (4x mode) + tree-structured tensor adds (2x) so
    only the final add depends on the PSUM evacuation.
  * All transposes on the PE (no DMA transposes); gram matmuls
    interleaved with the next group's transposes.
  * PSUM evacuations spread across Scalar/Vector/GpSimd engines.
  * Channels head-interleaved [q_h0,k_h0,...,q_h3,k_h3, v] so per-head
    gram operands are contiguous column slices of the transposed stripe.
"""

import numpy as np
import ml_dtypes

import concourse.bass as bass
import concourse.tile as tile
from concourse import mybir, bacc
from concourse.bass_utils import run_bass_kernel_spmd

F32 = mybir.dt.float32
BF16 = mybir.dt.bfloat16
FP8 = mybir.dt.float8e4
DR = mybir.MatmulPerfMode.DoubleRow
AX = mybir.AxisListType
OP = mybir.AluOpType
ACTF = mybir.ActivationFunctionType

C, H, W = 192, 128, 128
HW = H * W
HEADS, CH = 4, 48
RS = 132                      # padded row stride
TAPS = [(dy, dx) for dy in (-1, 0, 1) for dx in (-1, 0, 1)]
SA = 32
SB = 32
# per-chunk PE-tap assignment (rest go to the DVE)
CHUNK_PE_TAPS = [[0, 2, 6, 8, 3]] * 5
DIAG_OFF = [0]
for _t in CHUNK_PE_TAPS:
    DIAG_OFF.append(DIAG_OFF[-1] + len(_t))
N_DIAG = DIAG_OFF[-1]

_cached = {}


def _build_program():
    nc = bacc.Bacc("TRN2", target_bir_lowering=False, debug=False, num_devices=8)

    xq_d = nc.dram_tensor("xq", [96, 2, HW], FP8, kind="ExternalInput").ap()
    xb_d = nc.dram_tensor("xb", [C, H, W], BF16, kind="ExternalInput").ap()
    w1q_d = nc.dram_tensor("w1q", [96, 2, 576], FP8, kind="ExternalInput").ap()
    wvt_d = nc.dram_tensor("wvt", [C, 192], BF16, kind="ExternalInput").ap()
    dwt_d = nc.dram_tensor("dwt", [128, 5, 18], F32, kind="ExternalInput").ap()
    pwt_d = nc.dram_tensor("pwt", [48, 4, C], BF16, kind="ExternalInput").ap()
    i96_d = nc.dram_tensor("i96", [96, 96], F32, kind="ExternalInput").ap()
    i128_d = nc.dram_tensor("i128", [128, 128], BF16, kind="ExternalInput").ap()
    t4_d = nc.dram_tensor("t4", [1, 4], F32, kind="ExternalInput").ap()
    y_d = nc.dram_tensor("y", [C, HW], F32, kind="ExternalOutput").ap()

    with tile.TileContext(nc) as tc:
        _emit(nc, tc, xq_d, xb_d, w1q_d, wvt_d, dwt_d, pwt_d, i96_d,
              i128_d, t4_d, y_d)
    nc.finalize()
    return nc


def _emit(nc, tc, xq_d, xb_d, w1q_d, wvt_d, dwt_d, pwt_d, i96_d,
          i128_d, t4_d, y_d):
    from contextlib import ExitStack

    with ExitStack() as top:
        persist = top.enter_context(tc.tile_pool(name="persist", bufs=1))

        w1q = persist.tile([96, 2, 576], FP8)
        nc.sync.dma_start(out=w1q, in_=w1q_d)
        dwt = persist.tile([128, 5, 18], F32)
        nc.sync.dma_start(out=dwt, in_=dwt_d)
        i128 = persist.tile([128, 128], BF16)
        nc.sync.dma_start(out=i128, in_=i128_d)

        # first x stripe as early as possible (both halves on separate
        # queues), ahead of the weights only needed later
        xq = persist.tile([96, 2, HW], FP8)
        px0 = slice(0, SA * W)
        nc.sync.dma_start(out=xq[:, 0, px0], in_=xq_d[:, 0, px0])
        nc.gpsimd.dma_start(out=xq[:, 1, px0], in_=xq_d[:, 1, px0])

        wvt0 = persist.tile([128, 192], BF16)
        wvt1 = persist.tile([64, 192], BF16)
        nc.sync.dma_start(out=wvt0, in_=wvt_d[0:128, :])
        nc.sync.dma_start(out=wvt1, in_=wvt_d[128:192, :])
        pwt = persist.tile([48, 4, C], BF16)
        nc.sync.dma_start(out=pwt, in_=pwt_d)
        i96 = persist.tile([96, 96], F32)
        nc.sync.dma_start(out=i96, in_=i96_d)
        t4s = persist.tile([1, 4], F32)
        nc.sync.dma_start(out=t4s, in_=t4_d)
        # broadcast temperature to 48 partitions early (off critical path)
        t4b = persist.tile([48, HEADS], F32)
        nc.sync.dma_start(
            out=t4b,
            in_=bass.AP(tensor=t4_d.tensor, offset=t4_d.offset,
                        ap=[[0, 48], [1, HEADS]]))

        # per-(chunk, pe-tap) diagonal weight matrices
        diag = persist.tile([128, N_DIAG, 128], BF16)
        for ci in range(5):
            for j, t in enumerate(CHUNK_PE_TAPS[ci]):
                nc.vector.tensor_scalar_mul(diag[:, DIAG_OFF[ci] + j, :],
                                            i128, dwt[:, ci, t:t + 1])

        mta = persist.tile([128, C], BF16)
        mtb = persist.tile([64, C], BF16)

        gpool = top.enter_context(tc.tile_pool(name="gpool", bufs=1, space="PSUM"))
        g_ps = [gpool.tile([96, HEADS, 96], F32, name=f"gps{i}")
                for i in range(2)]

        def load_stripe(s):
            # split the two K-slabs across DMA queues to halve load latency
            px = slice(s * SA * W, (s + 1) * SA * W)
            nc.sync.dma_start(out=xq[:, 0, px], in_=xq_d[:, 0, px])
            nc.gpsimd.dma_start(out=xq[:, 1, px], in_=xq_d[:, 1, px])

        # ================= pass A: q,k =================
        with ExitStack() as pa:
            pre_p = pa.enter_context(tc.tile_pool(name="pre_p", bufs=4))
            tmp_p = pa.enter_context(tc.tile_pool(name="tmp_p", bufs=4))
            acc_p = pa.enter_context(tc.tile_pool(name="acc_p", bufs=4))
            qkt_p = pa.enter_context(tc.tile_pool(name="qkt_p", bufs=2))
            ps_p = pa.enter_context(tc.tile_pool(name="ps_p", bufs=2, space="PSUM"))
            cps_p = pa.enter_context(tc.tile_pool(name="cps_p", bufs=2, space="PSUM"))
            tps_p = pa.enter_context(tc.tile_pool(name="tps_p", bufs=2, space="PSUM"))

            for s in range(H // SA):
                if s + 1 < H // SA:
                    load_stripe(s + 1)
                _conv_stripe_qk(nc, s, SA, w1q, xq,
                                dwt, diag, i128,
                                pre_p, tmp_p, acc_p, qkt_p, ps_p, cps_p,
                                tps_p, g_ps, n_stripes=H // SA)

        # ============ pass B: v + output, finalize overlapped ============
        with ExitStack() as pb:
            xbs_p = pb.enter_context(tc.tile_pool(name="xbs_p", bufs=2))
            pre_p = pb.enter_context(tc.tile_pool(name="preb_p", bufs=2))
            tmp_p = pb.enter_context(tc.tile_pool(name="tmpb_p", bufs=4))
            acc_p = pb.enter_context(tc.tile_pool(name="accb_p", bufs=2))
            y_p = pb.enter_context(tc.tile_pool(name="y_p", bufs=3))
            ps_p = pb.enter_context(tc.tile_pool(name="psb_p", bufs=2, space="PSUM"))
            cps_p = pb.enter_context(tc.tile_pool(name="cpsb_p", bufs=2, space="PSUM"))

            def load_vstripe(s):
                r0 = s * SB
                lo, hi = max(r0 - 1, 0), min(r0 + SB + 1, H)
                xs0 = xbs_p.tile([128, SB + 2, W], BF16, tag="xs0", name="xs0")
                xs1 = xbs_p.tile([64, SB + 2, W], BF16, tag="xs1", name="xs1")
                nc.sync.dma_start(out=xs0[:, 0:hi - lo, :],
                                  in_=xb_d[0:128, lo:hi, :])
                nc.gpsimd.dma_start(out=xs1[:, 0:hi - lo, :],
                                    in_=xb_d[128:192, lo:hi, :])
                return (xs0, xs1)

            # stripe 0 v-production first, so the finalize (which only uses
            # ACT/DVE + tiny PE ops) overlaps with PE's qkv/tap work.
            xbs = {0: load_vstripe(0)}
            _conv_stripe_v_front(nc, 0, SB, wvt0, wvt1, xbs[0], dwt, diag,
                                 pre_p, tmp_p, acc_p, ps_p, cps_p,
                                 n_stripes=H // SB)
            xbs[1] = load_vstripe(1)

            with ExitStack() as fz:
                _finalize(nc, tc, fz, g_ps, i96, t4b, pwt, mta, mtb)

            yps_p = pb.enter_context(tc.tile_pool(name="yps_p", bufs=2, space="PSUM"))

            accs = {}
            for s in range(H // SB):
                if s > 0:
                    accs[s] = _conv_stripe_v_front(
                        nc, s, SB, wvt0, wvt1, xbs[s], dwt, diag,
                        pre_p, tmp_p, acc_p, ps_p, cps_p, n_stripes=H // SB)
                else:
                    accs[s] = _LAST_V_ACC[0]
                if s + 2 < H // SB:
                    xbs[s + 2] = load_vstripe(s + 2)
                _stripe_y(nc, s, SB, accs[s], mta, mtb, yps_p, y_p, y_d)


_LAST_V_ACC = [None]


def _pre_pads(nc, p, s, S, n_stripes, n_bufs):
    if s < n_bufs:  # pads once per pool buffer slot
        nc.gpsimd.memset(p[:, :, 0:2], 0.0)
        nc.gpsimd.memset(p[:, :, 130:132], 0.0)
    if s == 0:
        nc.gpsimd.memset(p[:, 0, :], 0.0)
    if s == n_stripes - 1:
        nc.gpsimd.memset(p[:, S + 1, :], 0.0)


def _stripe_matmul_dr(nc, s, S, n_stripes, oc_defs, w1q, xq,
                      pre, ps_p, n_bufs):
    """1x1 conv (fp8 DoubleRow, K=192 in one pass) into padded
    [p, S+2, RS] stripe tiles (halo rows included)."""
    r0 = s * S
    lo, hi = max(r0 - 1, 0), min(r0 + S + 1, H)
    for i, (ocp, ocsl) in enumerate(oc_defs):
        p = pre[i]
        _pre_pads(nc, p, s, S, n_stripes, n_bufs)
        # rows [lo, hi) of the image, in groups of <=4 rows (512 px)
        r = lo
        while r < hi:
            nr = min(4, hi - r)
            ps = ps_p.tile([ocp, 512], F32, tag="mmps", name="mmps")
            px = slice(r * W, (r + nr) * W)
            nc.tensor.matmul(ps[:, 0:nr * W], w1q[:, :, ocsl], xq[:, :, px],
                             start=True, stop=True, perf_mode=DR)
            tr = r - (r0 - 1)
            nc.scalar.copy(
                p[:, tr:tr + nr, 2:130],
                ps[:, 0:nr * W].rearrange("p (a b) -> p a b", b=W))
            r += nr


def _stripe_matmul_bf(nc, s, S, n_stripes, oc_defs, wvt0, wvt1, xbs,
                      pre, ps_p, n_bufs):
    """bf16 1x1 conv for the v chunks, from streamed x tiles
    (xbs = (xs0 [128, 34, W], xs1 [64, 34, W]) holding rows [lo, hi))."""
    r0 = s * S
    lo, hi = max(r0 - 1, 0), min(r0 + S + 1, H)
    xs0, xs1 = xbs
    for i, (ocp, ocsl) in enumerate(oc_defs):
        p = pre[i]
        _pre_pads(nc, p, s, S, n_stripes, n_bufs)
        r = lo
        while r < hi:
            nr = min(4, hi - r)
            ps = ps_p.tile([ocp, 512], F32, tag="mmps", name="mmps")
            rl = r - lo
            nc.tensor.matmul(ps[:, 0:nr * W], wvt0[:, ocsl],
                             xs0[:, rl:rl + nr, :], start=True, stop=False)
            nc.tensor.matmul(ps[:, 0:nr * W], wvt1[:, ocsl],
                             xs1[:, rl:rl + nr, :], start=False, stop=True)
            tr = r - (r0 - 1)
            nc.scalar.copy(
                p[:, tr:tr + nr, 2:130],
                ps[:, 0:nr * W].rearrange("p (a b) -> p a b", b=W))
            r += nr


def _dw_conv(nc, pre, tmp_p, acc, dwt, diag, oc_list, S, cps_p):
    """3x3 depthwise conv on padded [p, S+2, RS] tiles -> acc [p, S, W].
    Per-chunk PE taps run as diagonal matmuls into PSUM (evacuated as
    the accumulator init); DVE taps are 4x-mode muls + tree adds."""
    for i, oc in enumerate(oc_list):
        p, a = pre[i], acc[i]
        np_ = p.shape[0]
        pe_taps = CHUNK_PE_TAPS[oc]
        dve_taps = [t for t in range(9) if t not in pe_taps]
        npe = len(pe_taps)
        # process 4-row groups in pairs, interleaving the two PSUM banks:
        # same-bank matmuls are then 2 apart, which hides the
        # accumulate-to-same-address RAW between consecutive taps (the
        # same trick as the even/odd gram split). Also halves stationary
        # switches per matmul pair.
        for g2 in range(S // 8):
            cpA = cps_p.tile([np_, 512], F32, tag="cps", name="cpsA")
            cpB = cps_p.tile([np_, 512], F32, tag="cps", name="cpsB")
            for j, t in enumerate(pe_taps):
                dy, dx = TAPS[t]
                for cp, g in ((cpA, 2 * g2), (cpB, 2 * g2 + 1)):
                    rhs = p[:, 1 + dy + 4 * g: 1 + dy + 4 * g + 4,
                            2 + dx: 130 + dx]
                    nc.tensor.matmul(cp, diag[:np_, DIAG_OFF[oc] + j, :np_],
                                     rhs, start=(j == 0),
                                     stop=(j == npe - 1),
                                     skip_group_check=True)
            for cp, g in ((cpA, 2 * g2), (cpB, 2 * g2 + 1)):
                nc.scalar.copy(a[:, 4 * g:4 * g + 4, :],
                               cp.rearrange("p (a b) -> p a b", b=W))
        # DVE taps: muls into tmp tiles (4x mode), then a tree of adds so
        # only the final add depends on the PSUM evacuation above.
        tms = []
        for t in dve_taps:
            dy, dx = TAPS[t]
            wv = dwt[:np_, oc, t:t + 1]
            v = p[:, 1 + dy: 1 + dy + S, 2 + dx: 130 + dx]
            tm = tmp_p.tile([128, S, W], BF16, tag="tmp", name="tmp")
            nc.vector.tensor_scalar_mul(tm[:np_], v, wv)
            tms.append(tm[:np_])
        nc.vector.tensor_add(tms[0], tms[0], tms[1])
        nc.vector.tensor_add(tms[2], tms[2], tms[3])
        nc.vector.tensor_add(tms[0], tms[0], tms[2])
        if len(tms) == 5:
            nc.vector.tensor_add(tms[0], tms[0], tms[4])
        nc.vector.tensor_add(a, a, tms[0])


def _conv_stripe_qk(nc, s, S, w1q, xq, dwt, diag, i128,
                    pre_p, tmp_p, acc_p, qkt_p, ps_p, cps_p, tps_p, g_ps,
                    n_stripes):
    """q,k chunks, software-pipelined: the next chunk's qkv matmuls are
    interleaved between the previous chunk's tap pairs so the PE has tap
    work while the Scalar engine drains the qkv PSUM evacuations."""
    r0 = s * S
    lo, hi = max(r0 - 1, 0), min(r0 + S + 1, H)
    oc_sl = [slice(0, 128), slice(128, 256), slice(256, 384)]
    pre = [pre_p.tile([128, S + 2, RS], BF16, tag="pre", name=f"pre{i}")
           for i in range(3)]
    acc = [acc_p.tile([128, S, W], BF16, tag="acc", name=f"acc{i}")
           for i in range(3)]

    groups = []
    r = lo
    while r < hi:
        nr = min(4, hi - r)
        groups.append((r, nr))
        r += nr

    def qkv_groups(c, idx):
        for gi in idx:
            gr, nr = groups[gi]
            ps = ps_p.tile([128, 512], F32, tag="mmps", name="mmps")
            px = slice(gr * W, (gr + nr) * W)
            nc.tensor.matmul(ps[:, 0:nr * W], w1q[:, :, oc_sl[c]],
                             xq[:, :, px], start=True, stop=True,
                             perf_mode=DR)
            tr = gr - (r0 - 1)
            nc.scalar.copy(
                pre[c][:, tr:tr + nr, 2:130],
                ps[:, 0:nr * W].rearrange("p (a b) -> p a b", b=W))

    def tap_pair(c, g2):
        p, a = pre[c], acc[c]
        pe_taps = CHUNK_PE_TAPS[c]
        npe = len(pe_taps)
        cpA = cps_p.tile([128, 512], F32, tag="cps", name="cpsA")
        cpB = cps_p.tile([128, 512], F32, tag="cps", name="cpsB")
        for j, t in enumerate(pe_taps):
            dy, dx = TAPS[t]
            for cp, g in ((cpA, 2 * g2), (cpB, 2 * g2 + 1)):
                rhs = p[:, 1 + dy + 4 * g: 1 + dy + 4 * g + 4,
                        2 + dx: 130 + dx]
                nc.tensor.matmul(cp, diag[:, DIAG_OFF[c] + j, :], rhs,
                                 start=(j == 0), stop=(j == npe - 1),
                                 skip_group_check=True)
        for cp, g in ((cpA, 2 * g2), (cpB, 2 * g2 + 1)):
            nc.scalar.copy(a[:, 4 * g:4 * g + 4, :],
                           cp.rearrange("p (a b) -> p a b", b=W))

    def dve_taps(c):
        p, a = pre[c], acc[c]
        dts = [t for t in range(9) if t not in CHUNK_PE_TAPS[c]]
        tms = []
        for t in dts:
            dy, dx = TAPS[t]
            tm = tmp_p.tile([128, S, W], BF16, tag="tmp", name="tmp")
            nc.vector.tensor_scalar_mul(
                tm, p[:, 1 + dy: 1 + dy + S, 2 + dx: 130 + dx],
                dwt[:, c, t:t + 1])
            tms.append(tm)
        nc.vector.tensor_add(tms[0], tms[0], tms[1])
        nc.vector.tensor_add(tms[2], tms[2], tms[3])
        nc.vector.tensor_add(tms[0], tms[0], tms[2])
        nc.vector.tensor_add(a, a, tms[0])

    ng = len(groups)
    split = [(0, 3), (3, 5), (5, 7), (7, ng)]
    _pre_pads(nc, pre[0], s, S, n_stripes, 4)
    qkv_groups(0, range(ng))
    _pre_pads(nc, pre[1], s, S, n_stripes, 4)
    for g2 in range(S // 8):
        a0, a1 = split[g2]
        qkv_groups(1, range(a0, a1))
        tap_pair(0, g2)
    dve_taps(0)
    _pre_pads(nc, pre[2], s, S, n_stripes, 4)
    for g2 in range(S // 8):
        a0, a1 = split[g2]
        qkv_groups(2, range(a0, a1))
        tap_pair(1, g2)
    dve_taps(1)
    for g2 in range(S // 8):
        tap_pair(2, g2)
    dve_taps(2)

    # transpose each 128-px row to [px, ch]; interleave gram matmuls of
    # the previous 8-row group with this group's transposes.
    qkt = qkt_p.tile([128, S, 384], BF16)
    first = (s == 0)
    last = (s == n_stripes - 1)
    evac = [nc.scalar.copy, nc.vector.tensor_copy, nc.vector.tensor_copy]

    def emit_gram(g):
        for k in range(8):
            pc = 8 * g + k
            gp = g_ps[pc % 2]
            for h in range(HEADS):
                nc.tensor.matmul(
                    gp[:, h, :], qkt[:, pc, 96 * h:96 * h + 96],
                    qkt[:, pc, 96 * h:96 * h + 96],
                    start=(first and pc < 2),
                    stop=(last and pc >= S - 2),
                    skip_group_check=True)

    for g in range(S // 8):
        for i in range(3):
            tps = tps_p.tile([128, 8, 128], BF16, tag="tps", name="tps")
            for k in range(8):
                nc.tensor.transpose(tps[:, k, :], acc[i][:, 8 * g + k, :],
                                    i128)
            evac[i](qkt[:, 8 * g:8 * g + 8, 128 * i:128 * (i + 1)], tps)
        if g > 0:
            emit_gram(g - 1)
    emit_gram(S // 8 - 1)


def _conv_stripe_v_front(nc, s, S, wvt0, wvt1, xbs, dwt, diag,
                         pre_p, tmp_p, acc_p, ps_p, cps_p, n_stripes):
    """qkv 1x1 + depthwise for the v chunks of stripe s; returns acc."""
    pre = [pre_p.tile([128, S + 2, RS], BF16, tag="prev", name="prev0"),
           pre_p.tile([64, S + 2, RS], BF16, tag="prev1", name="prev1")]
    oc_defs = [(128, slice(0, 128)), (64, slice(128, 192))]
    _stripe_matmul_bf(nc, s, S, n_stripes, oc_defs, wvt0, wvt1, xbs,
                      pre, ps_p, n_bufs=2)

    acc = [acc_p.tile([128, S, W], BF16, tag="accv", name="accv0"),
           acc_p.tile([64, S, W], BF16, tag="accv1", name="accv1")]
    _dw_conv(nc, pre, tmp_p, acc, dwt, diag, [3, 4], S, cps_p)
    _LAST_V_ACC[0] = acc
    return acc


def _stripe_y(nc, s, S, acc, mta, mtb, yps_p, y_p, y_d):
    """y = M^T.T @ v  (attn+proj folded)"""
    r0 = s * S
    for g in range(S // 4):
        pxs = slice(4 * g, 4 * g + 4)
        dpx = slice(r0 * W + 512 * g, r0 * W + 512 * (g + 1))
        yp0 = yps_p.tile([128, 512], F32, tag="yp", name="yp0")
        nc.tensor.matmul(yp0, mta[:, 0:128], acc[0][:, pxs, :],
                         start=True, stop=False)
        nc.tensor.matmul(yp0, mtb[:, 0:128], acc[1][:, pxs, :],
                         start=False, stop=True)
        y0 = y_p.tile([128, 512], F32, tag="y0", name="y0")
        nc.scalar.copy(y0, yp0)
        nc.sync.dma_start(out=y_d[0:128, dpx], in_=y0)
        yp1 = yps_p.tile([64, 512], F32, tag="yp", name="yp1")
        nc.tensor.matmul(yp1, mta[:, 128:192], acc[0][:, pxs, :],
                         start=True, stop=False)
        nc.tensor.matmul(yp1, mtb[:, 128:192], acc[1][:, pxs, :],
                         start=False, stop=True)
        y1 = y_p.tile([64, 512], F32, tag="y1", name="y1")
        nc.scalar.copy(y1, yp1)
        nc.sync.dma_start(out=y_d[128:192, dpx], in_=y1)


def _finalize(nc, tc, fz, g_ps, i96, t4b, pwt, mta, mtb):
    """gram -> l2-normalized attention -> softmax -> folded M^T."""
    fpool = fz.enter_context(tc.tile_pool(name="fpool", bufs=1))
    fps = fz.enter_context(tc.tile_pool(name="fps", bufs=1, space="PSUM"))

    gs0 = fpool.tile([96, HEADS, 96], F32)
    nc.scalar.copy(gs0, g_ps[0])
    gs1 = fpool.tile([96, HEADS, 96], F32)
    nc.vector.tensor_copy(gs1, g_ps[1])
    gs = fpool.tile([96, HEADS, 96], F32)
    nc.vector.tensor_add(gs, gs0, gs1)

    i96b = bass.AP(tensor=i96.tensor, offset=i96.offset,
                   ap=[list(i96.ap[0]), [0, HEADS], [1, 96]])
    gdiag = fpool.tile([96, HEADS, 96], F32)
    nc.vector.tensor_mul(gdiag, gs, i96b)
    nrm2 = fpool.tile([96, HEADS], F32)
    nc.vector.reduce_sum(nrm2, gdiag, axis=AX.X)
    nrm = fpool.tile([96, HEADS], F32)
    nc.scalar.activation(nrm, nrm2, ACTF.Sqrt)
    nc.vector.tensor_scalar_max(nrm, nrm, 1e-12)
    rstd = fpool.tile([96, HEADS], F32)
    nc.vector.reciprocal(rstd, nrm)

    rq = fpool.tile([48, HEADS], F32)
    nc.vector.tensor_mul(rq, rstd[0:48, :], t4b)

    rkk = fpool.tile([48, HEADS], F32)
    nc.sync.dma_start(out=rkk, in_=rstd[48:96, :])
    rkps = fps.tile([4, 48], F32)
    nc.tensor.transpose(rkps, rkk, i96[0:48, 0:48])
    rkrow = fpool.tile([4, 48], F32)
    nc.vector.tensor_copy(rkrow, rkps)
    dram_p = fz.enter_context(tc.tile_pool(name="dram_p", bufs=1,
                                           space="DRAM"))
    rkd = dram_p.tile([4, 48], F32)
    nc.sync.dma_start(out=rkd, in_=rkrow)
    rk = fpool.tile([48, HEADS, 48], F32)
    for h in range(HEADS):
        bsrc = bass.AP(tensor=rkd.tensor,
                       offset=rkd.offset + h * 48,
                       ap=[[0, 48], [1, 48]])
        nc.sync.dma_start(out=rk[:, h, :], in_=bsrc)

    z = fpool.tile([48, HEADS, 48], F32)
    for h in range(HEADS):
        nc.vector.scalar_tensor_tensor(
            out=z[:, h, :], in0=gs[0:48, h, 48:96],
            scalar=rq[:, h:h + 1], in1=rk[:, h, :],
            op0=OP.mult, op1=OP.mult)
    mx = fpool.tile([48, HEADS], F32)
    nc.vector.reduce_max(mx, z, axis=AX.X)
    nmx = fpool.tile([48, HEADS], F32)
    nc.vector.tensor_scalar_mul(nmx, mx, -1.0)
    ez = fpool.tile([48, HEADS, 48], F32)
    for h in range(HEADS):
        nc.scalar.activation(ez[:, h, :], z[:, h, :], ACTF.Exp,
                             bias=nmx[:, h:h + 1], scale=1.0)
    sm = fpool.tile([48, HEADS], F32)
    nc.vector.reduce_sum(sm, ez, axis=AX.X)
    rs = fpool.tile([48, HEADS], F32)
    nc.vector.reciprocal(rs, sm)
    a_bf = fpool.tile([48, HEADS, 48], BF16)
    for h in range(HEADS):
        nc.vector.tensor_scalar_mul(a_bf[:, h, :], ez[:, h, :],
                                    rs[:, h:h + 1])

    m_bf = fpool.tile([48, HEADS, C], BF16)
    for h in range(HEADS):
        mps = fps.tile([48, C], F32, tag="mps", name="mps")
        nc.tensor.matmul(mps, a_bf[:, h, :], pwt[:, h, :],
                         start=True, stop=True)
        nc.scalar.copy(m_bf[:, h, :], mps)

    nc.sync.dma_start(out=mta[0:48, :], in_=m_bf[:, 0, :])
    nc.sync.dma_start(out=mta[48:96, :], in_=m_bf[:, 1, :])
    nc.sync.dma_start(out=mta[96:128, :], in_=m_bf[0:32, 2, :])
    nc.sync.dma_start(out=mtb[0:16, :], in_=m_bf[32:48, 2, :])
    nc.sync.dma_start(out=mtb[16:64, :], in_=m_bf[:, 3, :])


# ---------------- host glue ----------------

def _host_inputs(x, qkv_w, dw_w, proj_w, temperature):
    perm = []
    for h in range(HEADS):
        perm += list(range(h * CH, (h + 1) * CH))
        perm += list(range(C + h * CH, C + (h + 1) * CH))
    perm += list(range(2 * C, 3 * C))
    perm = np.array(perm)

    w1 = np.asarray(qkv_w)[perm]
    w1t = np.ascontiguousarray(w1.T).astype(np.float32)  # [192, 576]
    w1q = np.stack([w1t[0:96], w1t[96:192]], axis=1)     # [96, 2, 576]
    w1q = np.clip(w1q, -240.0, 240.0).astype(ml_dtypes.float8_e4m3)
    wvt = np.ascontiguousarray(w1t[:, 384:576]).astype(ml_dtypes.bfloat16)
    dw = np.asarray(dw_w)[perm, 0]
    dwt = np.zeros((128, 5, 18), np.float32)
    for ci in range(5):
        rows = min(128, 576 - ci * 128)
        taps = dw[ci * 128: ci * 128 + rows].reshape(rows, 9)
        dwt[:rows, ci, 0:9] = taps
        dwt[:rows, ci, 9:18] = -taps
    pT = np.asarray(proj_w).T.astype(np.float32)
    pwt = np.stack([pT[48 * h:48 * (h + 1)] for h in range(4)],
                   axis=1).astype(ml_dtypes.bfloat16)
    i96 = np.eye(96, dtype=np.float32)
    i128 = np.eye(128, dtype=ml_dtypes.bfloat16)
    t4 = np.asarray(temperature).reshape(1, HEADS).astype(np.float32)
    shared = {
        "w1q": w1q, "wvt": wvt, "dwt": dwt, "pwt": pwt, "i96": i96,
        "i128": i128, "t4": t4,
    }
    xs = np.asarray(x).reshape(8, C, HW).astype(np.float32)
    xqs = np.clip(xs, -240.0, 240.0)
    xqs = np.stack([xqs[:, 0:96], xqs[:, 96:192]], axis=2) \
        .astype(ml_dtypes.float8_e4m3)                   # [8, 96, 2, HW]
    xbs = xs.astype(ml_dtypes.bfloat16).reshape(8, C, H, W)
    return shared, xqs, xbs


def kernel(x, qkv_w, dw_w, proj_w, temperature, _trace=False):
    if "nc" not in _cached:
        _cached["nc"] = _build_program()
    nc = _cached["nc"]
    shared, xqs, xbs = _host_inputs(x, qkv_w, dw_w, proj_w, temperature)
    in_maps = [dict(shared, xq=np.ascontiguousarray(xqs[i]),
                    xb=np.ascontiguousarray(xbs[i]))
               for i in range(8)]
    res = run_bass_kernel_spmd(nc, in_maps, core_ids=list(range(8)),
                               trace=_trace)
    out = np.stack([np.asarray(res.results[i]["y"]).reshape(C, H, W)
                    for i in range(8)])
    if _trace:
        _cached["last_exec_time_ns"] = res.exec_time_ns
        _cached["last_results"] = res
    return out
